# revision 49
# baseline (speedup 1.0000x reference)
"""Multi-Head Latent Attention (MLA) Trainium2 kernel, 8-way sharded.

Sharding: 8 cores = 2 (batch) x 4 (head groups of 4 heads). Each core
handles one batch element and 4 of the 16 heads; host sums the 4 W_O
row-slice partials per batch element.

v3 design (vs the fp32r spill-to-DRAM baseline, ~1.45ms):
  * q path folded on HOST: W_eff = W_D_Q @ [W_U_Q_g | W_Q_R_g], so the
    per-core replicated qc = x @ W_D_Q (the single biggest matmul) is
    gone from the device entirely (-40% phase-A flops).
  * Everything runs in bf16 (1 cyc/row on the PE, same as fp32r, but
    half the SBUF/DMA bytes). fp8 was measured 7.5% off: in diffuse
    attention O is a weighted mean of V, so every relative error in
    V/P/scores passes straight through; bf16 lands ~0.5%.
  * Fully SBUF-resident: no DRAM spills. Phase-A weights live in a
    phase-A-scoped pool and are replaced by W_O/O^T in phase B, so the
    peak fits in SBUF. Total DMA ~45MB vs ~170MB.
  * Everything computed TRANSPOSED (features on partitions). The
    decoupled-RoPE halves of two adjacent heads stay stacked in one
    128-partition tile (even head on partitions 0:64, odd on 64:128);
    score matmuls address them via base_partition=64 APs, so no
    cross-partition moves exist anywhere.
  * Softmax: plain exp on the scalar engine (scores are O(+-2)),
    denominator via a ones-matmul accumulated next to O^T = V^T P^T.
  * The output projection is interleaved into the attention stream
    (two matmuls per score tile), so the PE never drains while the
    scalar engine catches up on exp and the DVE normalizes; a
    continuously-busy PE also holds the 2.4GHz pstate.
"""

import sys

sys.path.insert(0, "/opt/trn_rl_repo")

import numpy as np
import ml_dtypes

import concourse.bacc as bacc
import concourse.mybir as mybir
import concourse.tile as tile
from concourse.bass_utils import run_bass_kernel_spmd

# Problem dims (hardcoded per contract)
D, NH, DH, DC, DCQ, DHR = 2048, 16, 128, 512, 1536, 64
B, L = 2, 2048
ROPE_THETA = 10000.0

NHG = 4                 # heads per core
DQB = NHG * DH          # 512: per-core base q/k feature dim (also v dim)
DQR = NHG * DHR         # 256: per-core rope feature dim
P = 128
CW = 512                # phase-A token chunk width
LQ = 512                # phase-B query block
KD = D // P             # 16 contraction subtiles over D
KC = DC // P            # 4 contraction subtiles over DC
NCH = L // CW           # 4 token chunks
NLK = L // P            # 16 key chunks per query block
SCALE = DH ** -0.5

BF = mybir.dt.bfloat16
F8 = mybir.dt.float8e4
F32 = mybir.dt.float32
DR = mybir.MatmulPerfMode.DoubleRow
CP = mybir.ActivationFunctionType.Copy
EXP = mybir.ActivationFunctionType.Exp

_CACHED = {}


def _build(repeat=None):
    nc = bacc.Bacc("TRN2", target_bir_lowering=False, debug=False)

    # ---- DRAM I/O (per-core data; program is SPMD)
    x16 = nc.dram_tensor("x16", [P, KD, L], BF, kind="ExternalInput")
    wq16 = nc.dram_tensor("wq16", [P, KD, DQB + DQR], BF, kind="ExternalInput")
    wkv16 = nc.dram_tensor("wkv16", [P, KD, DC], BF, kind="ExternalInput")
    wkr16 = nc.dram_tensor("wkr16", [P, KD, DQR], BF, kind="ExternalInput")
    wuk16 = nc.dram_tensor("wuk16", [P, KC, DQB], BF, kind="ExternalInput")
    wuv16 = nc.dram_tensor("wuv16", [P, KC, DQB], BF, kind="ExternalInput")
    wo16 = nc.dram_tensor("wo16", [P, NHG, D], BF, kind="ExternalInput")
    cosr = nc.dram_tensor("cosr", [P, L], BF, kind="ExternalInput")
    sinr = nc.dram_tensor("sinr", [P, L], BF, kind="ExternalInput")
    protT = nc.dram_tensor("protT", [P, P], BF, kind="ExternalInput")
    out = nc.dram_tensor("out", [L, D], F32, kind="ExternalOutput")

    from contextlib import nullcontext
    with tile.TileContext(nc) as tc:
        with (tc.For_i(0, repeat, 1) if repeat else nullcontext()), \
             tc.tile_pool(name="persist", bufs=1) as pp:
            # Scores run in fp8 DoubleRow for heads 0-1 and bf16 for heads
            # 2-3: all-fp8 scores measured 2.18e-2 output error (just over
            # the 2e-2 budget); halving the fp8 share scales the score noise
            # by sqrt(1/2) -> ~1.55e-2 total, and still saves 27us of PE.
            #
            # fp8 packs (heads 0,1), DoubleRow layout: [:, 0, :] = base,
            # [:, 1, :] = rope. k-side sub1 carries BOTH stacked heads' rope
            # (0:64 even head, 64:128 odd); q-side sub1 is zero-padded at
            # the other head's half, so one 256-row pass = base+rope score.
            kp_sb = [pp.tile([P, 2, L], F8, name=f"kp{h}", tag=f"kp{h}")
                     for h in range(2)]
            qp_sb = [pp.tile([P, 2, L], F8, name=f"qp{h}", tag=f"qp{h}")
                     for h in range(2)]
            for h in range(2):
                pad = slice(DHR, P) if h % 2 == 0 else slice(0, DHR)
                nc.any.memset(qp_sb[h][pad, 1, :], 0.0)
            # bf16 operands (heads 2,3): separate base tiles, stacked k-rope
            # pair tile, and per-head zero-padded q-rope tiles (full 128-row
            # matmuls -> no PE tile-config switches).
            kb_sb = {h: pp.tile([P, L], BF, name=f"kb{h}", tag=f"kb{h}")
                     for h in (2, 3)}
            qb_sb = {h: pp.tile([P, L], BF, name=f"qb{h}", tag=f"qb{h}")
                     for h in (2, 3)}
            kr1_sb = pp.tile([P, L], BF, name="kr1_sb", tag="kr1")
            qr_sb = {h: pp.tile([P, L], BF, name=f"qr{h}", tag=f"qr{h}")
                     for h in (2, 3)}
            for h in (2, 3):
                pad = slice(DHR, P) if h % 2 == 0 else slice(0, DHR)
                nc.any.memset(qr_sb[h][pad, :], 0.0)
            v_sb = pp.tile([P, NLK, DQB], BF, name="v_sb", tag="v")

            # ================= Phase A: projections (token-chunked) =========
            with tc.tile_pool(name="wA", bufs=1) as wA, \
                 tc.tile_pool(name="xp", bufs=2) as xp, \
                 tc.tile_pool(name="csbp", bufs=2) as csbp, \
                 tc.tile_pool(name="ropp", bufs=2) as ropp, \
                 tc.tile_pool(name="rtmp", bufs=1) as rtmp, \
                 tc.tile_pool(name="psA", bufs=4, space="PSUM") as psA, \
                 tc.tile_pool(name="psR", bufs=2, space="PSUM") as psR:

                # DMA emission in first-use order: x chunk 0 + wkv (the
                # c-projection inputs) fire first so the PE starts early.
                def load_x(ch):
                    t = xp.tile([P, KD, CW], BF, name="xs", tag="xs")
                    nc.sync.dma_start(out=t[:],
                                      in_=x16[:, :, ch * CW:(ch + 1) * CW])
                    return t

                xs_t = [None] * NCH
                xs_t[0] = load_x(0)
                # wkv split per col-block: the first c matmul only needs
                # block 0, so the PE can start ~0.5MB in
                wkv_sb = wA.tile([P, KD, DC], BF, name="wkv_sb", tag="wkv")
                for m in range(KC):
                    nc.sync.dma_start(out=wkv_sb[:, :, m * P:(m + 1) * P],
                                      in_=wkv16[:, :, m * P:(m + 1) * P])
                wq_sb = wA.tile([P, KD, DQB + DQR], BF, name="wq_sb", tag="wq")
                nc.sync.dma_start(out=wq_sb[:], in_=wq16[:, :, :])
                cos_sb = wA.tile([P, L], BF, name="cos_sb", tag="cos")
                nc.sync.dma_start(out=cos_sb[:], in_=cosr[:, :])
                sin_sb = wA.tile([P, L], BF, name="sin_sb", tag="sin")
                nc.sync.dma_start(out=sin_sb[:], in_=sinr[:, :])
                prot_sb = wA.tile([P, P], BF, name="prot_sb", tag="prot")
                nc.sync.dma_start(out=prot_sb[:], in_=protT[:, :])
                wuk_sb = wA.tile([P, KC, DQB], BF, name="wuk_sb", tag="wuk")
                nc.sync.dma_start(out=wuk_sb[:], in_=wuk16[:, :, :])
                wuv_sb = wA.tile([P, KC, DQB], BF, name="wuv_sb", tag="wuv")
                nc.sync.dma_start(out=wuv_sb[:], in_=wuv16[:, :, :])
                wkr_sb = wA.tile([P, KD, DQR], BF, name="wkr_sb", tag="wkr")
                nc.sync.dma_start(out=wkr_sb[:], in_=wkr16[:, :, :])
                xs_t[1] = load_x(1)

                def proj(ps, w_sb, col, rhs, nk):
                    for k in range(nk):
                        nc.tensor.matmul(
                            ps[:], w_sb[:, k, col * P:(col + 1) * P],
                            rhs[:, k, :], start=(k == 0), stop=(k == nk - 1))

                def rope_pair(raw_ps, m, tsl, split):
                    """RoPE a [128, CW] raw psum tile (heads 2m, 2m+1 x 64
                    rope dims). split=None -> whole tile into kr_sb[m];
                    else write each head's half into its padded qr tile."""
                    raw = ropp.tile([P, CW], BF, name="raw", tag="raw")
                    nc.scalar.activation(raw[:], raw_ps[:], CP)
                    rps = psR.tile([P, CW], F32, name="rps", tag="rps")
                    nc.tensor.matmul(rps[:], prot_sb[:], raw[:],
                                     start=True, stop=True)
                    t1 = rtmp.tile([P, CW], BF, name="t1", tag="t1")
                    nc.vector.tensor_mul(t1[:], raw[:], cos_sb[:, tsl])
                    t2 = rtmp.tile([P, CW], BF, name="t2", tag="t2")
                    nc.vector.tensor_mul(t2[:], rps[:], sin_sb[:, tsl])
                    lo, hi = slice(0, DHR), slice(DHR, P)
                    if split is None:  # k-rope
                        if m == 0:  # fp8 heads: both tiles get the full pair
                            nc.vector.tensor_add(kp_sb[0][:, 1, tsl],
                                                 t1[:], t2[:])
                            nc.vector.tensor_add(kp_sb[1][:, 1, tsl],
                                                 t1[:], t2[:])
                        else:
                            nc.vector.tensor_add(kr1_sb[:, tsl], t1[:], t2[:])
                    elif m == 0:  # q-rope, fp8 heads
                        nc.vector.tensor_add(qp_sb[0][lo, 1, tsl],
                                             t1[lo, :], t2[lo, :])
                        nc.vector.tensor_add(qp_sb[1][hi, 1, tsl],
                                             t1[hi, :], t2[hi, :])
                    else:  # q-rope, bf16 heads
                        nc.vector.tensor_add(qr_sb[2][lo, tsl],
                                             t1[lo, :], t2[lo, :])
                        nc.vector.tensor_add(qr_sb[3][hi, tsl],
                                             t1[hi, :], t2[hi, :])

                for ch in range(NCH):
                    tsl = slice(ch * CW, (ch + 1) * CW)
                    xs = xs_t[ch]

                    # c^T latent slab (DC x CW)
                    c_sb = csbp.tile([P, KC, CW], BF, name="c_sb", tag="c")
                    for m in range(KC):
                        ps = psA.tile([P, CW], F32, name="ps_c", tag="psa")
                        proj(ps, wkv_sb, m, xs, KD)
                        nc.any.tensor_copy(c_sb[:, m, :], ps[:])

                    # q_base^T (4 head blocks) + q_rope^T (2 stacked blocks)
                    for m in range(NHG):
                        ps = psA.tile([P, CW], F32, name="ps_qb", tag="psa")
                        proj(ps, wq_sb, m, xs, KD)
                        dst = qp_sb[m][:, 0, tsl] if m < 2 else qb_sb[m][:, tsl]
                        nc.any.tensor_copy(dst, ps[:])
                    for m in range(DQR // P):
                        ps = psR.tile([P, CW], F32, name="ps_qr", tag="rps")
                        proj(ps, wq_sb, NHG + m, xs, KD)
                        rope_pair(ps, m, tsl, split=True)

                    # k_base^T (4 head blocks, from c)
                    for m in range(NHG):
                        ps = psA.tile([P, CW], F32, name="ps_kb", tag="psa")
                        proj(ps, wuk_sb, m, c_sb, KC)
                        dst = kp_sb[m][:, 0, tsl] if m < 2 else kb_sb[m][:, tsl]
                        nc.any.tensor_copy(dst, ps[:])

                    # v natural (CW tokens x DQB, from c)
                    for lt in range(CW // P):
                        ps = psA.tile([P, DQB], F32, name="ps_v", tag="psa")
                        for k in range(KC):
                            nc.tensor.matmul(
                                ps[:], c_sb[:, k, lt * P:(lt + 1) * P],
                                wuv_sb[:, k, :],
                                start=(k == 0), stop=(k == KC - 1))
                        nc.any.tensor_copy(v_sb[:, ch * (CW // P) + lt, :], ps[:])

                    # k_rope^T (2 stacked blocks)
                    for m in range(DQR // P):
                        ps = psR.tile([P, CW], F32, name="ps_kr", tag="rps")
                        proj(ps, wkr_sb, m, xs, KD)
                        rope_pair(ps, m, tsl, split=None)

                    if ch + 2 < NCH:
                        xs_t[ch + 2] = load_x(ch + 2)

            # ================= Phase B+C: attention + out-proj ==============
            with tc.tile_pool(name="wB", bufs=1) as wB, \
                 tc.tile_pool(name="ptp", bufs=5) as ptp, \
                 tc.tile_pool(name="recp", bufs=2) as recp, \
                 tc.tile_pool(name="ostg", bufs=3) as ostgp, \
                 tc.tile_pool(name="stp", bufs=3, space="PSUM") as stp, \
                 tc.tile_pool(name="otp", bufs=2, space="PSUM") as otp, \
                 tc.tile_pool(name="rsp", bufs=1, space="PSUM") as rsp, \
                 tc.tile_pool(name="psC", bufs=2, space="PSUM") as psC:

                wo_sb = wB.tile([P, NHG, D], BF, name="wo_sb", tag="wo")
                nc.sync.dma_start(out=wo_sb[:], in_=wo16[:, :, :])
                ones_sb = wB.tile([P, P], BF, name="ones_sb", tag="ones")
                nc.any.memset(ones_sb[:], 1.0)
                oT = [wB.tile([P, L], BF, name=f"oT{h}", tag=f"oT{h}")
                      for h in range(NHG)]

                def c_work(lq):
                    """Out-projection for query block lq, yielded one matmul
                    at a time so it can interleave into the attention."""
                    for mtl in range(LQ // P):
                        mt = lq * (LQ // P) + mtl
                        for nt in range(D // 512):
                            ps = psC.tile([P, 512], F32, name="ps_o", tag="psc")
                            for h in range(NHG):
                                nc.tensor.matmul(
                                    ps[:], oT[h][:, mt * P:(mt + 1) * P],
                                    wo_sb[:, h, nt * 512:(nt + 1) * 512],
                                    start=(h == 0), stop=(h == NHG - 1))
                                yield
                            stg = ostgp.tile([P, 512], F32, name="stg",
                                             tag="stg")
                            nc.vector.tensor_copy(stg[:], ps[:])
                            nc.sync.dma_start(
                                out=out[mt * P:(mt + 1) * P,
                                        nt * 512:(nt + 1) * 512],
                                in_=stg[:])

                def drain(gen, n):
                    if gen is not None:
                        for _ in range(n):
                            if next(gen, "done") == "done":
                                return None
                    return gen

                cg = None  # C-work generator for the previous query block
                for lq in range(L // LQ):
                    qsl = slice(lq * LQ, (lq + 1) * LQ)
                    for h in range(NHG):
                        ot_ps = otp.tile([P, LQ], F32, name="ot_ps", tag="ot")
                        rs_ps = rsp.tile([P, LQ], F32, name="rs_ps", tag="rs")

                        def scores(lk):
                            sp = stp.tile([P, LQ], F32, name="sp", tag="sp")
                            ksl = slice(lk * P, (lk + 1) * P)
                            if h < 2:
                                # one 256-row fp8 DoubleRow pass: base + rope
                                nc.tensor.matmul(
                                    sp[:], kp_sb[h][:, :, ksl],
                                    qp_sb[h][:, :, qsl],
                                    start=True, stop=True, perf_mode=DR)
                            else:
                                nc.tensor.matmul(
                                    sp[:], kb_sb[h][:, ksl],
                                    qb_sb[h][:, qsl], start=True, stop=False)
                                nc.tensor.matmul(
                                    sp[:], kr1_sb[:, ksl],
                                    qr_sb[h][:, qsl], start=False, stop=True)
                            pt = ptp.tile([P, LQ], BF, name="pt", tag="pt")
                            nc.scalar.activation(pt[:], sp[:], EXP, scale=SCALE)
                            return pt

                        # lead-2 software pipeline: exp latency (~800ns incl
                        # semaphores) hides under two slots of PE work
                        pt_q = [scores(0), scores(1)]
                        for lk in range(NLK):
                            if lk + 2 < NLK:
                                pt_q.append(scores(lk + 2))
                            pt = pt_q.pop(0)
                            nc.tensor.matmul(
                                ot_ps[:], v_sb[:, lk, h * DH:(h + 1) * DH],
                                pt[:], start=(lk == 0), stop=(lk == NLK - 1))
                            nc.tensor.matmul(
                                rs_ps[:], ones_sb[:], pt[:],
                                start=(lk == 0), stop=(lk == NLK - 1))
                            cg = drain(cg, 1)

                        rec = recp.tile([P, LQ], F32, name="rec", tag="rec")
                        nc.vector.reciprocal_approx_fast(out=rec[:], in_=rs_ps[:])
                        nc.vector.tensor_mul(oT[h][:, qsl], ot_ps[:], rec[:])
                    cg = drain(cg, 65)  # finish any straggler C work
                    cg = c_work(lq)
                cg = drain(cg, 65)

    nc.compile()
    return nc


def _rope_tables():
    """cos/sin in transposed, 2-head-replicated layout (128 x L), plus
    Prot^T (pair-swap rotation, block-diag over the 2 stacked heads)."""
    inv_freq = 1.0 / (ROPE_THETA ** (np.arange(0, DHR, 2, dtype=np.float32) / DHR))
    ang = np.arange(L, dtype=np.float32)[:, None] * inv_freq[None, :]  # (L, 32)
    cos64 = np.concatenate([np.cos(ang), np.cos(ang)], axis=1).T  # (64, L)
    sin64 = np.concatenate([np.sin(ang), np.sin(ang)], axis=1).T
    cosr = np.tile(cos64, (2, 1)).astype(ml_dtypes.bfloat16)
    sinr = np.tile(sin64, (2, 1)).astype(ml_dtypes.bfloat16)
    p64 = np.zeros((DHR, DHR), dtype=np.float32)
    half = DHR // 2
    p64[np.arange(half), np.arange(half) + half] = -1.0
    p64[np.arange(half) + half, np.arange(half)] = 1.0
    p128 = np.zeros((P, P), dtype=np.float32)
    p128[:DHR, :DHR] = p64
    p128[DHR:, DHR:] = p64
    protT = np.ascontiguousarray(p128.T).astype(ml_dtypes.bfloat16)
    return cosr, sinr, protT


def _ktiled(w, nk):
    """[nk*128, N] -> [128, nk, N] contraction-subtile layout, bf16."""
    n = w.shape[1]
    t = np.ascontiguousarray(
        np.asarray(w, dtype=np.float32).reshape(nk, P, n).transpose(1, 0, 2))
    return t.astype(ml_dtypes.bfloat16)


def _prepare_in_maps(x, W_D_Q, W_U_Q, W_Q_R, W_D_KV, W_U_K, W_K_R, W_U_V, W_O):
    cosr, sinr, protT = _rope_tables()
    x = np.asarray(x, dtype=np.float32)
    W_D_Q = np.asarray(W_D_Q, dtype=np.float32)
    x16s = [_ktiled(np.ascontiguousarray(x[b].T), KD) for b in range(B)]

    in_maps = []
    shared = {}
    for c in range(8):
        b, g = c // 4, c % 4
        if g not in shared:
            hb = slice(g * DQB, (g + 1) * DQB)
            hr = slice(g * DQR, (g + 1) * DQR)
            weff = W_D_Q @ np.concatenate(
                [np.asarray(W_U_Q, np.float32)[:, hb],
                 np.asarray(W_Q_R, np.float32)[:, hr]], axis=1)
            shared[g] = dict(
                wq16=_ktiled(weff, KD),
                wkv16=_ktiled(W_D_KV, KD),
                wkr16=_ktiled(np.asarray(W_K_R, np.float32)[:, hr], KD),
                wuk16=_ktiled(np.asarray(W_U_K, np.float32)[:, hb], KC),
                wuv16=_ktiled(np.asarray(W_U_V, np.float32)[:, hb], KC),
                wo16=_ktiled(np.asarray(W_O, np.float32)[hb, :], NHG),
                cosr=cosr, sinr=sinr, protT=protT,
            )
        in_maps.append(dict(x16=x16s[b], **shared[g]))
    return in_maps


def kernel(x, W_D_Q, W_U_Q, W_Q_R, W_D_KV, W_U_K, W_K_R, W_U_V, W_O):
    if "nc" not in _CACHED:
        _CACHED["nc"] = _build()
    nc = _CACHED["nc"]
    in_maps = _prepare_in_maps(x, W_D_Q, W_U_Q, W_Q_R, W_D_KV,
                               W_U_K, W_K_R, W_U_V, W_O)
    res = run_bass_kernel_spmd(nc, in_maps, core_ids=list(range(8)))
    outs = [r["out"] for r in res.results]
    full = np.stack(
        [outs[b * 4] + outs[b * 4 + 1] + outs[b * 4 + 2] + outs[b * 4 + 3]
         for b in range(B)]).astype(np.float32)
    return full


# revision 50
# speedup vs baseline: 1.0058x; 1.0058x over previous
"""Multi-Head Latent Attention (MLA) Trainium2 kernel, 8-way sharded.

Sharding: 8 cores = 2 (batch) x 4 (head groups of 4 heads). Each core
handles one batch element and 4 of the 16 heads; host sums the 4 W_O
row-slice partials per batch element.

v3 design (vs the fp32r spill-to-DRAM baseline, ~1.45ms):
  * q path folded on HOST: W_eff = W_D_Q @ [W_U_Q_g | W_Q_R_g], so the
    per-core replicated qc = x @ W_D_Q (the single biggest matmul) is
    gone from the device entirely (-40% phase-A flops).
  * Everything runs in bf16 (1 cyc/row on the PE, same as fp32r, but
    half the SBUF/DMA bytes). fp8 was measured 7.5% off: in diffuse
    attention O is a weighted mean of V, so every relative error in
    V/P/scores passes straight through; bf16 lands ~0.5%.
  * Fully SBUF-resident: no DRAM spills. Phase-A weights live in a
    phase-A-scoped pool and are replaced by W_O/O^T in phase B, so the
    peak fits in SBUF. Total DMA ~45MB vs ~170MB.
  * Everything computed TRANSPOSED (features on partitions). The
    decoupled-RoPE halves of two adjacent heads stay stacked in one
    128-partition tile (even head on partitions 0:64, odd on 64:128);
    score matmuls address them via base_partition=64 APs, so no
    cross-partition moves exist anywhere.
  * Softmax: plain exp on the scalar engine (scores are O(+-2)),
    denominator via a ones-matmul accumulated next to O^T = V^T P^T.
  * The output projection is interleaved into the attention stream
    (two matmuls per score tile), so the PE never drains while the
    scalar engine catches up on exp and the DVE normalizes; a
    continuously-busy PE also holds the 2.4GHz pstate.
"""

import sys

sys.path.insert(0, "/opt/trn_rl_repo")

import numpy as np
import ml_dtypes

import concourse.bacc as bacc
import concourse.mybir as mybir
import concourse.tile as tile
from concourse.bass_utils import run_bass_kernel_spmd

# Problem dims (hardcoded per contract)
D, NH, DH, DC, DCQ, DHR = 2048, 16, 128, 512, 1536, 64
B, L = 2, 2048
ROPE_THETA = 10000.0

NHG = 4                 # heads per core
DQB = NHG * DH          # 512: per-core base q/k feature dim (also v dim)
DQR = NHG * DHR         # 256: per-core rope feature dim
P = 128
CW = 512                # phase-A token chunk width
LQ = 512                # phase-B query block
KD = D // P             # 16 contraction subtiles over D
KC = DC // P            # 4 contraction subtiles over DC
NCH = L // CW           # 4 token chunks
NLK = L // P            # 16 key chunks per query block
SCALE = DH ** -0.5

BF = mybir.dt.bfloat16
F8 = mybir.dt.float8e4
F32 = mybir.dt.float32
DR = mybir.MatmulPerfMode.DoubleRow
CP = mybir.ActivationFunctionType.Copy
EXP = mybir.ActivationFunctionType.Exp

_CACHED = {}


def _build(repeat=None):
    nc = bacc.Bacc("TRN2", target_bir_lowering=False, debug=False)

    # ---- DRAM I/O (per-core data; program is SPMD)
    x16 = nc.dram_tensor("x16", [P, KD, L], BF, kind="ExternalInput")
    wq16 = nc.dram_tensor("wq16", [P, KD, DQB + DQR], BF, kind="ExternalInput")
    wkv16 = nc.dram_tensor("wkv16", [P, KD, DC], BF, kind="ExternalInput")
    wkr16 = nc.dram_tensor("wkr16", [P, KD, DQR], BF, kind="ExternalInput")
    wuk16 = nc.dram_tensor("wuk16", [P, KC, DQB], BF, kind="ExternalInput")
    wuv16 = nc.dram_tensor("wuv16", [P, KC, DQB], BF, kind="ExternalInput")
    wo16 = nc.dram_tensor("wo16", [P, NHG, D], BF, kind="ExternalInput")
    cosr = nc.dram_tensor("cosr", [P, L], BF, kind="ExternalInput")
    sinr = nc.dram_tensor("sinr", [P, L], BF, kind="ExternalInput")
    protT = nc.dram_tensor("protT", [P, P], BF, kind="ExternalInput")
    out = nc.dram_tensor("out", [L, D], F32, kind="ExternalOutput")

    from contextlib import nullcontext
    with tile.TileContext(nc) as tc:
        with (tc.For_i(0, repeat, 1) if repeat else nullcontext()), \
             tc.tile_pool(name="persist", bufs=1) as pp:
            # Scores run in fp8 DoubleRow for heads 0-1 and bf16 for heads
            # 2-3: all-fp8 scores measured 2.18e-2 output error (just over
            # the 2e-2 budget); halving the fp8 share scales the score noise
            # by sqrt(1/2) -> ~1.55e-2 total, and still saves 27us of PE.
            #
            # fp8 packs (heads 0,1), DoubleRow layout: [:, 0, :] = base,
            # [:, 1, :] = rope. k-side sub1 carries BOTH stacked heads' rope
            # (0:64 even head, 64:128 odd); q-side sub1 is zero-padded at
            # the other head's half, so one 256-row pass = base+rope score.
            kp_sb = [pp.tile([P, 2, L], F8, name=f"kp{h}", tag=f"kp{h}")
                     for h in range(2)]
            qp_sb = [pp.tile([P, 2, L], F8, name=f"qp{h}", tag=f"qp{h}")
                     for h in range(2)]
            for h in range(2):
                pad = slice(DHR, P) if h % 2 == 0 else slice(0, DHR)
                nc.any.memset(qp_sb[h][pad, 1, :], 0.0)
            # bf16 operands (heads 2,3): separate base tiles, stacked k-rope
            # pair tile, and per-head zero-padded q-rope tiles (full 128-row
            # matmuls -> no PE tile-config switches).
            kb_sb = {h: pp.tile([P, L], BF, name=f"kb{h}", tag=f"kb{h}")
                     for h in (2, 3)}
            qb_sb = {h: pp.tile([P, L], BF, name=f"qb{h}", tag=f"qb{h}")
                     for h in (2, 3)}
            kr1_sb = pp.tile([P, L], BF, name="kr1_sb", tag="kr1")
            qr_sb = {h: pp.tile([P, L], BF, name=f"qr{h}", tag=f"qr{h}")
                     for h in (2, 3)}
            for h in (2, 3):
                pad = slice(DHR, P) if h % 2 == 0 else slice(0, DHR)
                nc.any.memset(qr_sb[h][pad, :], 0.0)
            v_sb = pp.tile([P, NLK, DQB], BF, name="v_sb", tag="v")

            # ================= Phase A: projections (token-chunked) =========
            with tc.tile_pool(name="wA", bufs=1) as wA, \
                 tc.tile_pool(name="xp", bufs=2) as xp, \
                 tc.tile_pool(name="csbp", bufs=2) as csbp, \
                 tc.tile_pool(name="ropp", bufs=2) as ropp, \
                 tc.tile_pool(name="rtmp", bufs=1) as rtmp, \
                 tc.tile_pool(name="psA", bufs=4, space="PSUM") as psA, \
                 tc.tile_pool(name="psR", bufs=2, space="PSUM") as psR:

                # DMA emission in first-use order: x chunk 0 + wkv (the
                # c-projection inputs) fire first so the PE starts early.
                def load_x(ch):
                    t = xp.tile([P, KD, CW], BF, name="xs", tag="xs")
                    nc.sync.dma_start(out=t[:],
                                      in_=x16[:, :, ch * CW:(ch + 1) * CW])
                    return t

                xs_t = [None] * NCH
                xs_t[0] = load_x(0)
                wkv_sb = wA.tile([P, KD, DC], BF, name="wkv_sb", tag="wkv")
                nc.sync.dma_start(out=wkv_sb[:], in_=wkv16[:, :, :])
                wq_sb = wA.tile([P, KD, DQB + DQR], BF, name="wq_sb", tag="wq")
                nc.sync.dma_start(out=wq_sb[:], in_=wq16[:, :, :])
                cos_sb = wA.tile([P, L], BF, name="cos_sb", tag="cos")
                nc.sync.dma_start(out=cos_sb[:], in_=cosr[:, :])
                sin_sb = wA.tile([P, L], BF, name="sin_sb", tag="sin")
                nc.sync.dma_start(out=sin_sb[:], in_=sinr[:, :])
                prot_sb = wA.tile([P, P], BF, name="prot_sb", tag="prot")
                nc.sync.dma_start(out=prot_sb[:], in_=protT[:, :])
                wuk_sb = wA.tile([P, KC, DQB], BF, name="wuk_sb", tag="wuk")
                nc.sync.dma_start(out=wuk_sb[:], in_=wuk16[:, :, :])
                wuv_sb = wA.tile([P, KC, DQB], BF, name="wuv_sb", tag="wuv")
                nc.sync.dma_start(out=wuv_sb[:], in_=wuv16[:, :, :])
                wkr_sb = wA.tile([P, KD, DQR], BF, name="wkr_sb", tag="wkr")
                nc.sync.dma_start(out=wkr_sb[:], in_=wkr16[:, :, :])
                xs_t[1] = load_x(1)

                def proj(ps, w_sb, col, rhs, nk):
                    for k in range(nk):
                        nc.tensor.matmul(
                            ps[:], w_sb[:, k, col * P:(col + 1) * P],
                            rhs[:, k, :], start=(k == 0), stop=(k == nk - 1))

                def rope_pair(raw_ps, m, tsl, split):
                    """RoPE a [128, CW] raw psum tile (heads 2m, 2m+1 x 64
                    rope dims). split=None -> whole tile into kr_sb[m];
                    else write each head's half into its padded qr tile."""
                    raw = ropp.tile([P, CW], BF, name="raw", tag="raw")
                    nc.scalar.activation(raw[:], raw_ps[:], CP)
                    rps = psR.tile([P, CW], F32, name="rps", tag="rps")
                    nc.tensor.matmul(rps[:], prot_sb[:], raw[:],
                                     start=True, stop=True)
                    t1 = rtmp.tile([P, CW], BF, name="t1", tag="t1")
                    nc.vector.tensor_mul(t1[:], raw[:], cos_sb[:, tsl])
                    t2 = rtmp.tile([P, CW], BF, name="t2", tag="t2")
                    nc.vector.tensor_mul(t2[:], rps[:], sin_sb[:, tsl])
                    lo, hi = slice(0, DHR), slice(DHR, P)
                    if split is None:  # k-rope
                        if m == 0:  # fp8 heads: both tiles get the full pair
                            nc.vector.tensor_add(kp_sb[0][:, 1, tsl],
                                                 t1[:], t2[:])
                            nc.vector.tensor_add(kp_sb[1][:, 1, tsl],
                                                 t1[:], t2[:])
                        else:
                            nc.vector.tensor_add(kr1_sb[:, tsl], t1[:], t2[:])
                    elif m == 0:  # q-rope, fp8 heads
                        nc.vector.tensor_add(qp_sb[0][lo, 1, tsl],
                                             t1[lo, :], t2[lo, :])
                        nc.vector.tensor_add(qp_sb[1][hi, 1, tsl],
                                             t1[hi, :], t2[hi, :])
                    else:  # q-rope, bf16 heads
                        nc.vector.tensor_add(qr_sb[2][lo, tsl],
                                             t1[lo, :], t2[lo, :])
                        nc.vector.tensor_add(qr_sb[3][hi, tsl],
                                             t1[hi, :], t2[hi, :])

                for ch in range(NCH):
                    tsl = slice(ch * CW, (ch + 1) * CW)
                    xs = xs_t[ch]

                    # c^T latent slab (DC x CW)
                    c_sb = csbp.tile([P, KC, CW], BF, name="c_sb", tag="c")
                    for m in range(KC):
                        ps = psA.tile([P, CW], F32, name="ps_c", tag="psa")
                        proj(ps, wkv_sb, m, xs, KD)
                        nc.any.tensor_copy(c_sb[:, m, :], ps[:])

                    # q_base^T (4 head blocks) + q_rope^T (2 stacked blocks)
                    for m in range(NHG):
                        ps = psA.tile([P, CW], F32, name="ps_qb", tag="psa")
                        proj(ps, wq_sb, m, xs, KD)
                        dst = qp_sb[m][:, 0, tsl] if m < 2 else qb_sb[m][:, tsl]
                        nc.any.tensor_copy(dst, ps[:])
                    for m in range(DQR // P):
                        ps = psR.tile([P, CW], F32, name="ps_qr", tag="rps")
                        proj(ps, wq_sb, NHG + m, xs, KD)
                        rope_pair(ps, m, tsl, split=True)

                    # k_base^T (4 head blocks, from c)
                    for m in range(NHG):
                        ps = psA.tile([P, CW], F32, name="ps_kb", tag="psa")
                        proj(ps, wuk_sb, m, c_sb, KC)
                        dst = kp_sb[m][:, 0, tsl] if m < 2 else kb_sb[m][:, tsl]
                        nc.any.tensor_copy(dst, ps[:])

                    # v natural (CW tokens x DQB, from c)
                    for lt in range(CW // P):
                        ps = psA.tile([P, DQB], F32, name="ps_v", tag="psa")
                        for k in range(KC):
                            nc.tensor.matmul(
                                ps[:], c_sb[:, k, lt * P:(lt + 1) * P],
                                wuv_sb[:, k, :],
                                start=(k == 0), stop=(k == KC - 1))
                        nc.any.tensor_copy(v_sb[:, ch * (CW // P) + lt, :], ps[:])

                    # k_rope^T (2 stacked blocks)
                    for m in range(DQR // P):
                        ps = psR.tile([P, CW], F32, name="ps_kr", tag="rps")
                        proj(ps, wkr_sb, m, xs, KD)
                        rope_pair(ps, m, tsl, split=None)

                    if ch + 2 < NCH:
                        xs_t[ch + 2] = load_x(ch + 2)

            # ================= Phase B+C: attention + out-proj ==============
            with tc.tile_pool(name="wB", bufs=1) as wB, \
                 tc.tile_pool(name="ptp", bufs=5) as ptp, \
                 tc.tile_pool(name="recp", bufs=2) as recp, \
                 tc.tile_pool(name="ostg", bufs=2) as ostgp, \
                 tc.tile_pool(name="stp", bufs=3, space="PSUM") as stp, \
                 tc.tile_pool(name="otp", bufs=2, space="PSUM") as otp, \
                 tc.tile_pool(name="rsp", bufs=1, space="PSUM") as rsp, \
                 tc.tile_pool(name="psC", bufs=2, space="PSUM") as psC:

                wo_sb = wB.tile([P, NHG, D], BF, name="wo_sb", tag="wo")
                nc.sync.dma_start(out=wo_sb[:], in_=wo16[:, :, :])
                ones_sb = wB.tile([P, P], BF, name="ones_sb", tag="ones")
                nc.any.memset(ones_sb[:], 1.0)
                oT = [wB.tile([P, L], BF, name=f"oT{h}", tag=f"oT{h}")
                      for h in range(NHG)]

                def c_work(lq):
                    """Out-projection for query block lq, yielded one matmul
                    at a time so it can interleave into the attention."""
                    for mtl in range(LQ // P):
                        mt = lq * (LQ // P) + mtl
                        stg = ostgp.tile([P, D], F32, name="stg", tag="stg")
                        for nt in range(D // 512):
                            ps = psC.tile([P, 512], F32, name="ps_o", tag="psc")
                            for h in range(NHG):
                                nc.tensor.matmul(
                                    ps[:], oT[h][:, mt * P:(mt + 1) * P],
                                    wo_sb[:, h, nt * 512:(nt + 1) * 512],
                                    start=(h == 0), stop=(h == NHG - 1))
                                yield
                            nc.vector.tensor_copy(
                                stg[:, nt * 512:(nt + 1) * 512], ps[:])
                        nc.sync.dma_start(out=out[mt * P:(mt + 1) * P, :],
                                          in_=stg[:])

                def drain(gen, n):
                    if gen is not None:
                        for _ in range(n):
                            if next(gen, "done") == "done":
                                return None
                    return gen

                cg = None  # C-work generator for the previous query block
                for lq in range(L // LQ):
                    qsl = slice(lq * LQ, (lq + 1) * LQ)
                    for h in range(NHG):
                        ot_ps = otp.tile([P, LQ], F32, name="ot_ps", tag="ot")
                        rs_ps = rsp.tile([P, LQ], F32, name="rs_ps", tag="rs")

                        def scores(lk):
                            sp = stp.tile([P, LQ], F32, name="sp", tag="sp")
                            ksl = slice(lk * P, (lk + 1) * P)
                            if h < 2:
                                # one 256-row fp8 DoubleRow pass: base + rope
                                nc.tensor.matmul(
                                    sp[:], kp_sb[h][:, :, ksl],
                                    qp_sb[h][:, :, qsl],
                                    start=True, stop=True, perf_mode=DR)
                            else:
                                nc.tensor.matmul(
                                    sp[:], kb_sb[h][:, ksl],
                                    qb_sb[h][:, qsl], start=True, stop=False)
                                nc.tensor.matmul(
                                    sp[:], kr1_sb[:, ksl],
                                    qr_sb[h][:, qsl], start=False, stop=True)
                            pt = ptp.tile([P, LQ], BF, name="pt", tag="pt")
                            nc.scalar.activation(pt[:], sp[:], EXP, scale=SCALE)
                            return pt

                        # lead-2 software pipeline: exp latency (~800ns incl
                        # semaphores) hides under two slots of PE work
                        pt_q = [scores(0), scores(1)]
                        for lk in range(NLK):
                            if lk + 2 < NLK:
                                pt_q.append(scores(lk + 2))
                            pt = pt_q.pop(0)
                            nc.tensor.matmul(
                                ot_ps[:], v_sb[:, lk, h * DH:(h + 1) * DH],
                                pt[:], start=(lk == 0), stop=(lk == NLK - 1))
                            nc.tensor.matmul(
                                rs_ps[:], ones_sb[:], pt[:],
                                start=(lk == 0), stop=(lk == NLK - 1))
                            cg = drain(cg, 1)

                        rec = recp.tile([P, LQ], F32, name="rec", tag="rec")
                        nc.vector.reciprocal_approx_fast(out=rec[:], in_=rs_ps[:])
                        nc.vector.tensor_mul(oT[h][:, qsl], ot_ps[:], rec[:])
                    cg = drain(cg, 65)  # finish any straggler C work
                    cg = c_work(lq)
                cg = drain(cg, 65)

    nc.compile()
    return nc


def _rope_tables():
    """cos/sin in transposed, 2-head-replicated layout (128 x L), plus
    Prot^T (pair-swap rotation, block-diag over the 2 stacked heads)."""
    inv_freq = 1.0 / (ROPE_THETA ** (np.arange(0, DHR, 2, dtype=np.float32) / DHR))
    ang = np.arange(L, dtype=np.float32)[:, None] * inv_freq[None, :]  # (L, 32)
    cos64 = np.concatenate([np.cos(ang), np.cos(ang)], axis=1).T  # (64, L)
    sin64 = np.concatenate([np.sin(ang), np.sin(ang)], axis=1).T
    cosr = np.tile(cos64, (2, 1)).astype(ml_dtypes.bfloat16)
    sinr = np.tile(sin64, (2, 1)).astype(ml_dtypes.bfloat16)
    p64 = np.zeros((DHR, DHR), dtype=np.float32)
    half = DHR // 2
    p64[np.arange(half), np.arange(half) + half] = -1.0
    p64[np.arange(half) + half, np.arange(half)] = 1.0
    p128 = np.zeros((P, P), dtype=np.float32)
    p128[:DHR, :DHR] = p64
    p128[DHR:, DHR:] = p64
    protT = np.ascontiguousarray(p128.T).astype(ml_dtypes.bfloat16)
    return cosr, sinr, protT


def _ktiled(w, nk):
    """[nk*128, N] -> [128, nk, N] contraction-subtile layout, bf16."""
    n = w.shape[1]
    t = np.ascontiguousarray(
        np.asarray(w, dtype=np.float32).reshape(nk, P, n).transpose(1, 0, 2))
    return t.astype(ml_dtypes.bfloat16)


def _prepare_in_maps(x, W_D_Q, W_U_Q, W_Q_R, W_D_KV, W_U_K, W_K_R, W_U_V, W_O):
    cosr, sinr, protT = _rope_tables()
    x = np.asarray(x, dtype=np.float32)
    W_D_Q = np.asarray(W_D_Q, dtype=np.float32)
    x16s = [_ktiled(np.ascontiguousarray(x[b].T), KD) for b in range(B)]

    in_maps = []
    shared = {}
    for c in range(8):
        b, g = c // 4, c % 4
        if g not in shared:
            hb = slice(g * DQB, (g + 1) * DQB)
            hr = slice(g * DQR, (g + 1) * DQR)
            weff = W_D_Q @ np.concatenate(
                [np.asarray(W_U_Q, np.float32)[:, hb],
                 np.asarray(W_Q_R, np.float32)[:, hr]], axis=1)
            shared[g] = dict(
                wq16=_ktiled(weff, KD),
                wkv16=_ktiled(W_D_KV, KD),
                wkr16=_ktiled(np.asarray(W_K_R, np.float32)[:, hr], KD),
                wuk16=_ktiled(np.asarray(W_U_K, np.float32)[:, hb], KC),
                wuv16=_ktiled(np.asarray(W_U_V, np.float32)[:, hb], KC),
                wo16=_ktiled(np.asarray(W_O, np.float32)[hb, :], NHG),
                cosr=cosr, sinr=sinr, protT=protT,
            )
        in_maps.append(dict(x16=x16s[b], **shared[g]))
    return in_maps


def kernel(x, W_D_Q, W_U_Q, W_Q_R, W_D_KV, W_U_K, W_K_R, W_U_V, W_O):
    if "nc" not in _CACHED:
        _CACHED["nc"] = _build()
    nc = _CACHED["nc"]
    in_maps = _prepare_in_maps(x, W_D_Q, W_U_Q, W_Q_R, W_D_KV,
                               W_U_K, W_K_R, W_U_V, W_O)
    res = run_bass_kernel_spmd(nc, in_maps, core_ids=list(range(8)))
    outs = [r["out"] for r in res.results]
    full = np.stack(
        [outs[b * 4] + outs[b * 4 + 1] + outs[b * 4 + 2] + outs[b * 4 + 3]
         for b in range(B)]).astype(np.float32)
    return full


# revision 53
# speedup vs baseline: 1.0111x; 1.0053x over previous
"""Multi-Head Latent Attention (MLA) Trainium2 kernel, 8-way sharded.

Sharding: 8 cores = 2 (batch) x 4 (head groups of 4 heads). Each core
handles one batch element and 4 of the 16 heads; host sums the 4 W_O
row-slice partials per batch element.

v3 design (vs the fp32r spill-to-DRAM baseline, ~1.45ms):
  * q path folded on HOST: W_eff = W_D_Q @ [W_U_Q_g | W_Q_R_g], so the
    per-core replicated qc = x @ W_D_Q (the single biggest matmul) is
    gone from the device entirely (-40% phase-A flops).
  * Everything runs in bf16 (1 cyc/row on the PE, same as fp32r, but
    half the SBUF/DMA bytes). fp8 was measured 7.5% off: in diffuse
    attention O is a weighted mean of V, so every relative error in
    V/P/scores passes straight through; bf16 lands ~0.5%.
  * Fully SBUF-resident: no DRAM spills. Phase-A weights live in a
    phase-A-scoped pool and are replaced by W_O/O^T in phase B, so the
    peak fits in SBUF. Total DMA ~45MB vs ~170MB.
  * Everything computed TRANSPOSED (features on partitions). The
    decoupled-RoPE halves of two adjacent heads stay stacked in one
    128-partition tile (even head on partitions 0:64, odd on 64:128);
    score matmuls address them via base_partition=64 APs, so no
    cross-partition moves exist anywhere.
  * Softmax: plain exp on the scalar engine (scores are O(+-2)),
    denominator via a ones-matmul accumulated next to O^T = V^T P^T.
  * The output projection is interleaved into the attention stream
    (two matmuls per score tile), so the PE never drains while the
    scalar engine catches up on exp and the DVE normalizes; a
    continuously-busy PE also holds the 2.4GHz pstate.
"""

import sys

sys.path.insert(0, "/opt/trn_rl_repo")

import numpy as np
import ml_dtypes

import concourse.bacc as bacc
import concourse.mybir as mybir
import concourse.tile as tile
from concourse.bass_utils import run_bass_kernel_spmd

# Problem dims (hardcoded per contract)
D, NH, DH, DC, DCQ, DHR = 2048, 16, 128, 512, 1536, 64
B, L = 2, 2048
ROPE_THETA = 10000.0

NHG = 4                 # heads per core
DQB = NHG * DH          # 512: per-core base q/k feature dim (also v dim)
DQR = NHG * DHR         # 256: per-core rope feature dim
P = 128
CW = 512                # phase-A token chunk width
LQ = 512                # phase-B query block
KD = D // P             # 16 contraction subtiles over D
KC = DC // P            # 4 contraction subtiles over DC
NCH = L // CW           # 4 token chunks
NLK = L // P            # 16 key chunks per query block
SCALE = DH ** -0.5

BF = mybir.dt.bfloat16
F8 = mybir.dt.float8e4
F32 = mybir.dt.float32
DR = mybir.MatmulPerfMode.DoubleRow
CP = mybir.ActivationFunctionType.Copy
EXP = mybir.ActivationFunctionType.Exp

_CACHED = {}


def _build(repeat=None):
    nc = bacc.Bacc("TRN2", target_bir_lowering=False, debug=False)

    # ---- DRAM I/O (per-core data; program is SPMD)
    x16 = nc.dram_tensor("x16", [P, KD, L], BF, kind="ExternalInput")
    wq16 = nc.dram_tensor("wq16", [P, KD, DQB + DQR], BF, kind="ExternalInput")
    wkv16 = nc.dram_tensor("wkv16", [P, KD, DC], BF, kind="ExternalInput")
    wkr16 = nc.dram_tensor("wkr16", [P, KD, DQR], BF, kind="ExternalInput")
    wuk16 = nc.dram_tensor("wuk16", [P, KC, DQB], BF, kind="ExternalInput")
    wuv16 = nc.dram_tensor("wuv16", [P, KC, DQB], BF, kind="ExternalInput")
    wo16 = nc.dram_tensor("wo16", [P, NHG, D], BF, kind="ExternalInput")
    cosr = nc.dram_tensor("cosr", [P, L], BF, kind="ExternalInput")
    sinr = nc.dram_tensor("sinr", [P, L], BF, kind="ExternalInput")
    protT = nc.dram_tensor("protT", [P, P], BF, kind="ExternalInput")
    out = nc.dram_tensor("out", [L, D], F32, kind="ExternalOutput")

    from contextlib import nullcontext
    with tile.TileContext(nc) as tc:
        with (tc.For_i(0, repeat, 1) if repeat else nullcontext()), \
             tc.tile_pool(name="persist", bufs=1) as pp:
            # Scores run in fp8 DoubleRow for heads 0-1 and bf16 for heads
            # 2-3: all-fp8 scores measured 2.18e-2 output error (just over
            # the 2e-2 budget); halving the fp8 share scales the score noise
            # by sqrt(1/2) -> ~1.55e-2 total, and still saves 27us of PE.
            #
            # fp8 packs (heads 0,1), DoubleRow layout: [:, 0, :] = base,
            # [:, 1, :] = rope. k-side sub1 carries BOTH stacked heads' rope
            # (0:64 even head, 64:128 odd); q-side sub1 is zero-padded at
            # the other head's half, so one 256-row pass = base+rope score.
            kp_sb = [pp.tile([P, 2, L], F8, name=f"kp{h}", tag=f"kp{h}")
                     for h in range(2)]
            qp_sb = [pp.tile([P, 2, L], F8, name=f"qp{h}", tag=f"qp{h}")
                     for h in range(2)]
            for h in range(2):
                pad = slice(DHR, P) if h % 2 == 0 else slice(0, DHR)
                nc.any.memset(qp_sb[h][pad, 1, :], 0.0)
            # bf16 operands (heads 2,3): separate base tiles, stacked k-rope
            # pair tile, and per-head zero-padded q-rope tiles (full 128-row
            # matmuls -> no PE tile-config switches).
            kb_sb = {h: pp.tile([P, L], BF, name=f"kb{h}", tag=f"kb{h}")
                     for h in (2, 3)}
            qb_sb = {h: pp.tile([P, L], BF, name=f"qb{h}", tag=f"qb{h}")
                     for h in (2, 3)}
            kr1_sb = pp.tile([P, L], BF, name="kr1_sb", tag="kr1")
            qr_sb = {h: pp.tile([P, L], BF, name=f"qr{h}", tag=f"qr{h}")
                     for h in (2, 3)}
            for h in (2, 3):
                pad = slice(DHR, P) if h % 2 == 0 else slice(0, DHR)
                nc.any.memset(qr_sb[h][pad, :], 0.0)
            v_sb = pp.tile([P, NLK, DQB], BF, name="v_sb", tag="v")

            # ================= Phase A: projections (token-chunked) =========
            with tc.tile_pool(name="wA", bufs=1) as wA, \
                 tc.tile_pool(name="xp", bufs=2) as xp, \
                 tc.tile_pool(name="csbp", bufs=2) as csbp, \
                 tc.tile_pool(name="ropp", bufs=2) as ropp, \
                 tc.tile_pool(name="rtmp", bufs=1) as rtmp, \
                 tc.tile_pool(name="psA", bufs=4, space="PSUM") as psA, \
                 tc.tile_pool(name="psR", bufs=2, space="PSUM") as psR:

                # DMA emission in first-use order: x chunk 0 + wkv (the
                # c-projection inputs) fire first so the PE starts early.
                def load_x(ch):
                    t = xp.tile([P, KD, CW], BF, name="xs", tag="xs")
                    nc.sync.dma_start(out=t[:],
                                      in_=x16[:, :, ch * CW:(ch + 1) * CW])
                    return t

                xs_t = [None] * NCH
                xs_t[0] = load_x(0)
                # wkv split per col-block: the first c matmul chain only
                # needs block 0, so the PE starts ~0.5MB of weights in
                wkv_sb = wA.tile([P, KD, DC], BF, name="wkv_sb", tag="wkv")
                for m in range(KC):
                    nc.sync.dma_start(out=wkv_sb[:, :, m * P:(m + 1) * P],
                                      in_=wkv16[:, :, m * P:(m + 1) * P])
                wq_sb = wA.tile([P, KD, DQB + DQR], BF, name="wq_sb", tag="wq")
                nc.sync.dma_start(out=wq_sb[:], in_=wq16[:, :, :])
                cos_sb = wA.tile([P, L], BF, name="cos_sb", tag="cos")
                nc.sync.dma_start(out=cos_sb[:], in_=cosr[:, :])
                sin_sb = wA.tile([P, L], BF, name="sin_sb", tag="sin")
                nc.sync.dma_start(out=sin_sb[:], in_=sinr[:, :])
                prot_sb = wA.tile([P, P], BF, name="prot_sb", tag="prot")
                nc.sync.dma_start(out=prot_sb[:], in_=protT[:, :])
                wuk_sb = wA.tile([P, KC, DQB], BF, name="wuk_sb", tag="wuk")
                nc.sync.dma_start(out=wuk_sb[:], in_=wuk16[:, :, :])
                wuv_sb = wA.tile([P, KC, DQB], BF, name="wuv_sb", tag="wuv")
                nc.sync.dma_start(out=wuv_sb[:], in_=wuv16[:, :, :])
                wkr_sb = wA.tile([P, KD, DQR], BF, name="wkr_sb", tag="wkr")
                nc.sync.dma_start(out=wkr_sb[:], in_=wkr16[:, :, :])
                xs_t[1] = load_x(1)

                def proj(ps, w_sb, col, rhs, nk):
                    for k in range(nk):
                        nc.tensor.matmul(
                            ps[:], w_sb[:, k, col * P:(col + 1) * P],
                            rhs[:, k, :], start=(k == 0), stop=(k == nk - 1))

                def rope_pair(raw_ps, m, tsl, split):
                    """RoPE a [128, CW] raw psum tile (heads 2m, 2m+1 x 64
                    rope dims). split=None -> whole tile into kr_sb[m];
                    else write each head's half into its padded qr tile."""
                    raw = ropp.tile([P, CW], BF, name="raw", tag="raw")
                    nc.scalar.activation(raw[:], raw_ps[:], CP)
                    rps = psR.tile([P, CW], F32, name="rps", tag="rps")
                    nc.tensor.matmul(rps[:], prot_sb[:], raw[:],
                                     start=True, stop=True)
                    t1 = rtmp.tile([P, CW], BF, name="t1", tag="t1")
                    nc.vector.tensor_mul(t1[:], raw[:], cos_sb[:, tsl])
                    t2 = rtmp.tile([P, CW], BF, name="t2", tag="t2")
                    nc.vector.tensor_mul(t2[:], rps[:], sin_sb[:, tsl])
                    lo, hi = slice(0, DHR), slice(DHR, P)
                    if split is None:  # k-rope
                        if m == 0:  # fp8 heads: both tiles get the full pair
                            nc.vector.tensor_add(kp_sb[0][:, 1, tsl],
                                                 t1[:], t2[:])
                            nc.vector.tensor_add(kp_sb[1][:, 1, tsl],
                                                 t1[:], t2[:])
                        else:
                            nc.vector.tensor_add(kr1_sb[:, tsl], t1[:], t2[:])
                    elif m == 0:  # q-rope, fp8 heads
                        nc.vector.tensor_add(qp_sb[0][lo, 1, tsl],
                                             t1[lo, :], t2[lo, :])
                        nc.vector.tensor_add(qp_sb[1][hi, 1, tsl],
                                             t1[hi, :], t2[hi, :])
                    else:  # q-rope, bf16 heads
                        nc.vector.tensor_add(qr_sb[2][lo, tsl],
                                             t1[lo, :], t2[lo, :])
                        nc.vector.tensor_add(qr_sb[3][hi, tsl],
                                             t1[hi, :], t2[hi, :])

                for ch in range(NCH):
                    tsl = slice(ch * CW, (ch + 1) * CW)
                    xs = xs_t[ch]

                    # c^T latent slab (DC x CW)
                    c_sb = csbp.tile([P, KC, CW], BF, name="c_sb", tag="c")
                    for m in range(KC):
                        ps = psA.tile([P, CW], F32, name="ps_c", tag="psa")
                        proj(ps, wkv_sb, m, xs, KD)
                        nc.any.tensor_copy(c_sb[:, m, :], ps[:])

                    def qb_block(m):
                        ps = psA.tile([P, CW], F32, name="ps_qb", tag="psa")
                        proj(ps, wq_sb, m, xs, KD)
                        dst = qp_sb[m][:, 0, tsl] if m < 2 else qb_sb[m][:, tsl]
                        nc.any.tensor_copy(dst, ps[:])

                    def qr_block(m):
                        ps = psR.tile([P, CW], F32, name="ps_qr", tag="rps")
                        proj(ps, wq_sb, NHG + m, xs, KD)
                        rope_pair(ps, m, tsl, split=True)

                    def kb_block(m):
                        ps = psA.tile([P, CW], F32, name="ps_kb", tag="psa")
                        proj(ps, wuk_sb, m, c_sb, KC)
                        dst = kp_sb[m][:, 0, tsl] if m < 2 else kb_sb[m][:, tsl]
                        nc.any.tensor_copy(dst, ps[:])

                    def kr_block(m):
                        ps = psR.tile([P, CW], F32, name="ps_kr", tag="rps")
                        proj(ps, wkr_sb, m, xs, KD)
                        rope_pair(ps, m, tsl, split=None)

                    # heads 0-1's score operands first (phase B starts with
                    # head 0, so the last chunk's tail feeds it directly),
                    # v last (first consumed ~12 score-slots into phase B)
                    for m in (0, 1):
                        kb_block(m)
                    for m in (0, 1):
                        qb_block(m)
                    qr_block(0)
                    kr_block(0)
                    for m in (2, 3):
                        kb_block(m)
                    for m in (2, 3):
                        qb_block(m)
                    qr_block(1)
                    kr_block(1)

                    # v natural (CW tokens x DQB, from c)
                    for lt in range(CW // P):
                        ps = psA.tile([P, DQB], F32, name="ps_v", tag="psa")
                        for k in range(KC):
                            nc.tensor.matmul(
                                ps[:], c_sb[:, k, lt * P:(lt + 1) * P],
                                wuv_sb[:, k, :],
                                start=(k == 0), stop=(k == KC - 1))
                        nc.any.tensor_copy(v_sb[:, ch * (CW // P) + lt, :], ps[:])

                    if ch + 2 < NCH:
                        xs_t[ch + 2] = load_x(ch + 2)

            # ================= Phase B+C: attention + out-proj ==============
            with tc.tile_pool(name="wB", bufs=1) as wB, \
                 tc.tile_pool(name="ptp", bufs=5) as ptp, \
                 tc.tile_pool(name="recp", bufs=2) as recp, \
                 tc.tile_pool(name="ostg", bufs=2) as ostgp, \
                 tc.tile_pool(name="stp", bufs=3, space="PSUM") as stp, \
                 tc.tile_pool(name="otp", bufs=2, space="PSUM") as otp, \
                 tc.tile_pool(name="rsp", bufs=1, space="PSUM") as rsp, \
                 tc.tile_pool(name="psC", bufs=2, space="PSUM") as psC:

                wo_sb = wB.tile([P, NHG, D], BF, name="wo_sb", tag="wo")
                nc.sync.dma_start(out=wo_sb[:], in_=wo16[:, :, :])
                ones_sb = wB.tile([P, P], BF, name="ones_sb", tag="ones")
                nc.any.memset(ones_sb[:], 1.0)
                oT = [wB.tile([P, L], BF, name=f"oT{h}", tag=f"oT{h}")
                      for h in range(NHG)]

                def c_work(lq):
                    """Out-projection for query block lq, yielded one matmul
                    at a time so it can interleave into the attention."""
                    for mtl in range(LQ // P):
                        mt = lq * (LQ // P) + mtl
                        final = (mt == L // P - 1)
                        stg = ostgp.tile([P, D], F32, name="stg", tag="stg")
                        for nt in range(D // 512):
                            ps = psC.tile([P, 512], F32, name="ps_o", tag="psc")
                            for h in range(NHG):
                                nc.tensor.matmul(
                                    ps[:], oT[h][:, mt * P:(mt + 1) * P],
                                    wo_sb[:, h, nt * 512:(nt + 1) * 512],
                                    start=(h == 0), stop=(h == NHG - 1))
                                yield
                            nsl = slice(nt * 512, (nt + 1) * 512)
                            nc.vector.tensor_copy(stg[:, nsl], ps[:])
                            if final:  # drip the last block out per-nt so the
                                # kernel doesn't idle on one big tail DMA
                                nc.sync.dma_start(
                                    out=out[mt * P:(mt + 1) * P, nsl],
                                    in_=stg[:, nsl])
                        if not final:
                            nc.sync.dma_start(out=out[mt * P:(mt + 1) * P, :],
                                              in_=stg[:])

                def drain(gen, n):
                    if gen is not None:
                        for _ in range(n):
                            if next(gen, "done") == "done":
                                return None
                    return gen

                cg = None  # C-work generator for the previous query block
                for lq in range(L // LQ):
                    qsl = slice(lq * LQ, (lq + 1) * LQ)
                    for h in range(NHG):
                        ot_ps = otp.tile([P, LQ], F32, name="ot_ps", tag="ot")
                        rs_ps = rsp.tile([P, LQ], F32, name="rs_ps", tag="rs")

                        def scores(lk):
                            sp = stp.tile([P, LQ], F32, name="sp", tag="sp")
                            ksl = slice(lk * P, (lk + 1) * P)
                            if h < 2:
                                # one 256-row fp8 DoubleRow pass: base + rope
                                nc.tensor.matmul(
                                    sp[:], kp_sb[h][:, :, ksl],
                                    qp_sb[h][:, :, qsl],
                                    start=True, stop=True, perf_mode=DR)
                            else:
                                nc.tensor.matmul(
                                    sp[:], kb_sb[h][:, ksl],
                                    qb_sb[h][:, qsl], start=True, stop=False)
                                nc.tensor.matmul(
                                    sp[:], kr1_sb[:, ksl],
                                    qr_sb[h][:, qsl], start=False, stop=True)
                            pt = ptp.tile([P, LQ], BF, name="pt", tag="pt")
                            nc.scalar.activation(pt[:], sp[:], EXP, scale=SCALE)
                            return pt

                        # lead-2 software pipeline: exp latency (~800ns incl
                        # semaphores) hides under two slots of PE work
                        pt_q = [scores(0), scores(1)]
                        for lk in range(NLK):
                            if lk + 2 < NLK:
                                pt_q.append(scores(lk + 2))
                            pt = pt_q.pop(0)
                            nc.tensor.matmul(
                                ot_ps[:], v_sb[:, lk, h * DH:(h + 1) * DH],
                                pt[:], start=(lk == 0), stop=(lk == NLK - 1))
                            nc.tensor.matmul(
                                rs_ps[:], ones_sb[:], pt[:],
                                start=(lk == 0), stop=(lk == NLK - 1))
                            cg = drain(cg, 1)

                        rec = recp.tile([P, LQ], F32, name="rec", tag="rec")
                        nc.vector.reciprocal_approx_fast(out=rec[:], in_=rs_ps[:])
                        nc.vector.tensor_mul(oT[h][:, qsl], ot_ps[:], rec[:])
                    cg = drain(cg, 65)  # finish any straggler C work
                    cg = c_work(lq)
                cg = drain(cg, 65)

    nc.compile()
    return nc


def _rope_tables():
    """cos/sin in transposed, 2-head-replicated layout (128 x L), plus
    Prot^T (pair-swap rotation, block-diag over the 2 stacked heads)."""
    inv_freq = 1.0 / (ROPE_THETA ** (np.arange(0, DHR, 2, dtype=np.float32) / DHR))
    ang = np.arange(L, dtype=np.float32)[:, None] * inv_freq[None, :]  # (L, 32)
    cos64 = np.concatenate([np.cos(ang), np.cos(ang)], axis=1).T  # (64, L)
    sin64 = np.concatenate([np.sin(ang), np.sin(ang)], axis=1).T
    cosr = np.tile(cos64, (2, 1)).astype(ml_dtypes.bfloat16)
    sinr = np.tile(sin64, (2, 1)).astype(ml_dtypes.bfloat16)
    p64 = np.zeros((DHR, DHR), dtype=np.float32)
    half = DHR // 2
    p64[np.arange(half), np.arange(half) + half] = -1.0
    p64[np.arange(half) + half, np.arange(half)] = 1.0
    p128 = np.zeros((P, P), dtype=np.float32)
    p128[:DHR, :DHR] = p64
    p128[DHR:, DHR:] = p64
    protT = np.ascontiguousarray(p128.T).astype(ml_dtypes.bfloat16)
    return cosr, sinr, protT


def _ktiled(w, nk):
    """[nk*128, N] -> [128, nk, N] contraction-subtile layout, bf16."""
    n = w.shape[1]
    t = np.ascontiguousarray(
        np.asarray(w, dtype=np.float32).reshape(nk, P, n).transpose(1, 0, 2))
    return t.astype(ml_dtypes.bfloat16)


def _prepare_in_maps(x, W_D_Q, W_U_Q, W_Q_R, W_D_KV, W_U_K, W_K_R, W_U_V, W_O):
    cosr, sinr, protT = _rope_tables()
    x = np.asarray(x, dtype=np.float32)
    W_D_Q = np.asarray(W_D_Q, dtype=np.float32)
    x16s = [_ktiled(np.ascontiguousarray(x[b].T), KD) for b in range(B)]

    in_maps = []
    shared = {}
    for c in range(8):
        b, g = c // 4, c % 4
        if g not in shared:
            hb = slice(g * DQB, (g + 1) * DQB)
            hr = slice(g * DQR, (g + 1) * DQR)
            weff = W_D_Q @ np.concatenate(
                [np.asarray(W_U_Q, np.float32)[:, hb],
                 np.asarray(W_Q_R, np.float32)[:, hr]], axis=1)
            shared[g] = dict(
                wq16=_ktiled(weff, KD),
                wkv16=_ktiled(W_D_KV, KD),
                wkr16=_ktiled(np.asarray(W_K_R, np.float32)[:, hr], KD),
                wuk16=_ktiled(np.asarray(W_U_K, np.float32)[:, hb], KC),
                wuv16=_ktiled(np.asarray(W_U_V, np.float32)[:, hb], KC),
                wo16=_ktiled(np.asarray(W_O, np.float32)[hb, :], NHG),
                cosr=cosr, sinr=sinr, protT=protT,
            )
        in_maps.append(dict(x16=x16s[b], **shared[g]))
    return in_maps


def kernel(x, W_D_Q, W_U_Q, W_Q_R, W_D_KV, W_U_K, W_K_R, W_U_V, W_O):
    if "nc" not in _CACHED:
        _CACHED["nc"] = _build()
    nc = _CACHED["nc"]
    in_maps = _prepare_in_maps(x, W_D_Q, W_U_Q, W_Q_R, W_D_KV,
                               W_U_K, W_K_R, W_U_V, W_O)
    res = run_bass_kernel_spmd(nc, in_maps, core_ids=list(range(8)))
    outs = [r["out"] for r in res.results]
    full = np.stack(
        [outs[b * 4] + outs[b * 4 + 1] + outs[b * 4 + 2] + outs[b * 4 + 3]
         for b in range(B)]).astype(np.float32)
    return full
